# revision 4
# baseline (speedup 1.0000x reference)
"""Bass/Trainium2 kernel for nn_DefaultSegmentLinear (fp8 segment linear).

Reference semantics (CHUNKS=4, seg_mode='weight'):
    xq = e4m3fn(clip(x / in_scale, +-448))          # OCP e4m3, max 448
    wq = e4m3fn(clip(w_c / w_scales[c], +-448))     # per out-chunk of 1024
    out = (xq @ wq_c^T) * in_scale * w_scales[c] + bias

Sharding: 4-way over the 16384 tokens x 2-way over the 4096 out
features (8 cores; core cid -> token quarter q=cid//2, out half
h=cid%2).

Quantization happens host-side, exactly on the reference's grid: round
to OCP e4m3fn (the reference's own quantize), then halve and re-round
to TRN e4m3 (max 240).  Every OCP-e4m3 point v <= 448 has v/2 <= 224
exactly representable in TRN e4m3 (up to deep subnormals, identical to
the on-device halved-quantize this replaces), and the 4x is folded into
the output scale alpha_c = 4*in_scale*w_scales[c].  The device then
runs pure double-pumped fp8 matmuls (perf_mode=DoubleRow, K=256 per
instruction) -- no on-device quantize pass, and input DMA drops from
96 MB f32 to 24 MB fp8 per core.

Per-core dataflow (contraction i on partitions for both operands):
    x6  [TT=8, 128, KT=16, 2, NT=512] fp8: token-block-major so the
        first matmul group only needs a 2 MB slice (~6 us) instead of
        the whole 16 MB of x -- kills the startup bubble.
    w5  [OT=16, 128, KT, 2, 128] fp8: all 16 o-tile weights resident in
        SBUF (64 KB/partition).
    outT [2048, 4096] f32 (o, t); host transposes back.

Loop: for each token block tt (512 tokens = 1 PSUM bank), for each
o-tile: 16 DoubleRow matmuls accumulate K=4096, then one DVE
tensor_scalar (psum*alpha + bias) and a DMA out.  PSUM banks rotate
across o-tiles so the PE never stalls.
"""

import os

import ml_dtypes
import numpy as np

import concourse.bacc as bacc
import concourse.mybir as mybir
from concourse import tile
from concourse.bass_utils import run_bass_kernel_spmd

N_CORES = 8
TOKEN_WAYS, OUT_WAYS = 4, 2
B, S, IN, OUT = 4, 4096, 4096, 4096
TOK = B * S
T = TOK // TOKEN_WAYS    # 4096 tokens per core
OUT_C = OUT // OUT_WAYS  # 2048 out features per core
KT = IN // 256           # 16 contraction super-tiles (256 = 128 x 2)
OT = OUT_C // 128        # 16 out-feature tiles per core
NT = 512                 # moving free dim per matmul (one PSUM bank of f32)
TT = T // NT             # 8 token blocks
CHUNKS = 4
CHUNKS_C = CHUNKS // OUT_WAYS  # 2 weight chunks per core
OT_PER_CHUNK = OT // CHUNKS_C  # 8

F32 = mybir.dt.float32
FP8 = mybir.dt.float8e4

E4M3FN = ml_dtypes.float8_e4m3fn  # OCP: max 448 (reference grid)
E4M3 = ml_dtypes.float8_e4m3      # IEEE/TRN: max 240

_CACHE = {}


def _build():
    key = "nc"
    if key in _CACHE:
        return _CACHE[key]
    nc = bacc.Bacc(None, target_bir_lowering=False)
    x6 = nc.dram_tensor("x6", [TT, 128, KT, 2, NT], FP8, kind="ExternalInput")
    w5 = nc.dram_tensor("w5", [OT, 128, KT, 2, 128], FP8, kind="ExternalInput")
    biasv = nc.dram_tensor("biasv", [OUT_C], F32, kind="ExternalInput")
    alpha = nc.dram_tensor("alpha", [CHUNKS_C], F32, kind="ExternalInput")
    outT = nc.dram_tensor("outT", [OUT_C, T], F32, kind="ExternalOutput")

    DR = mybir.MatmulPerfMode.DoubleRow

    with tile.TileContext(nc) as tc:
        with (
            tc.tile_pool(name="consts", bufs=1) as consts,
            tc.tile_pool(name="wq", bufs=1) as wqp,
            tc.tile_pool(name="xt", bufs=3) as xtp,
            tc.tile_pool(name="osb", bufs=4) as osbp,
            tc.tile_pool(name="psum", bufs=8, space="PSUM") as psp,
        ):
            # DMA order matters: the first matmul group gates on wq0 +
            # the first half of x block 0, so those descriptors go
            # first.  x block 0 is split in two so the k=0..7 matmuls
            # can start before the k=8..15 half lands.
            wq = [
                wqp.tile([128, KT, 2, 128], FP8, tag=f"wq{ot}", name=f"wq{ot}")
                for ot in range(OT)
            ]
            nc.sync.dma_start(out=wq[0][:], in_=w5[0])
            xts = [None] * TT
            xts[0] = xtp.tile([128, KT, 2, NT], FP8, tag="xt", name="xt0")
            nc.sync.dma_start(out=xts[0][:, : KT // 2], in_=x6[0, :, : KT // 2])
            nc.sync.dma_start(out=xts[0][:, KT // 2 :], in_=x6[0, :, KT // 2 :])
            for ot in range(1, OT):
                nc.sync.dma_start(out=wq[ot][:], in_=w5[ot])

            al_b = []
            for c in range(CHUNKS_C):
                t = consts.tile([128, 1], F32, tag=f"al{c}")
                nc.sync.dma_start(
                    out=t[:], in_=alpha[c : c + 1].to_broadcast((128, 1))
                )
                al_b.append(t)
            bias_sb = consts.tile([128, OT], F32, tag="bias")
            nc.sync.dma_start(
                out=bias_sb[:], in_=biasv[:].rearrange("(j p) -> p j", p=128)
            )

            # Warm-up: the PE's HAM clock gate needs ~3.4 us of
            # sustained busy to lift the 1.2 GHz cold throttle.  Run
            # garbage matmuls (wq0 against itself, scratch PSUM bank)
            # gated only on wq0's DMA so the array is at 2.4 GHz when
            # the first real group issues.
            warm_ps = psp.tile([128, NT], F32, tag="ps", name="warm")
            for i in range(48):
                nc.tensor.matmul(
                    warm_ps[:, :128],
                    lhsT=wq[0][:, 0, 0, :],
                    rhs=wq[0][:, 0, 0, :],
                    start=True,
                    stop=True,
                )

            for tt in range(TT):
                if xts[tt] is None:
                    xts[tt] = xtp.tile(
                        [128, KT, 2, NT], FP8, tag="xt", name=f"xt{tt}"
                    )
                    nc.sync.dma_start(out=xts[tt][:], in_=x6[tt])
                xt = xts[tt]
                # Prefetch next x block right after this one's DMA slot.
                nxt = tt + 1
                if nxt < TT and xts[nxt] is None:
                    xts[nxt] = xtp.tile(
                        [128, KT, 2, NT], FP8, tag="xt", name=f"xt{nxt}"
                    )
                    nc.sync.dma_start(out=xts[nxt][:], in_=x6[nxt])
                for ot in range(OT):
                    c = ot // OT_PER_CHUNK
                    ps = psp.tile([128, NT], F32, tag="ps", name=f"ps{tt}_{ot}")
                    for k in range(KT):
                        nc.tensor.matmul(
                            ps[:],
                            lhsT=wq[ot][:, k, :, :],
                            rhs=xt[:, k, :, :],
                            start=(k == 0),
                            stop=(k == KT - 1),
                            perf_mode=DR,
                        )
                    ob = osbp.tile([128, NT], F32, tag="osb", name=f"ob{tt}_{ot}")
                    nc.vector.tensor_scalar(
                        ob[:],
                        ps[:],
                        al_b[c][:],
                        bias_sb[:, ot : ot + 1],
                        op0=mybir.AluOpType.mult,
                        op1=mybir.AluOpType.add,
                    )
                    nc.sync.dma_start(
                        out=outT[
                            128 * ot : 128 * (ot + 1), NT * tt : NT * (tt + 1)
                        ],
                        in_=ob[:],
                    )
    nc.compile()
    _CACHE[key] = nc
    return nc


def _quant_trn(a):
    """Reference-grid quantize to TRN e4m3 at half scale.

    Round to OCP e4m3fn exactly as the reference does, then halve
    (exact in f32) and round to TRN/IEEE e4m3.  The second rounding is
    the identity except for deep subnormals (same as the on-device
    halved quantize this replaces)."""
    q = np.clip(a, -448.0, 448.0).astype(E4M3FN)
    return (q.astype(np.float32) * np.float32(0.5)).astype(E4M3)


def prepare_in_maps(x, w, bias, in_scale, w_scales):
    """Host-side prep: scale normalization, fp8 quantize, layout."""
    assert x.shape == (B, S, IN) and w.shape == (OUT, IN)
    x = np.ascontiguousarray(x, dtype=np.float32)
    w = np.ascontiguousarray(w, dtype=np.float32)
    bias = np.ascontiguousarray(bias, dtype=np.float32)
    in_scale = np.float32(np.asarray(in_scale).reshape(()))
    w_scales = np.asarray(w_scales, dtype=np.float32).reshape(CHUNKS)

    xq8 = _quant_trn(x.reshape(TOK, IN) / in_scale)
    wn = (w.reshape(CHUNKS, OUT // CHUNKS, IN) / w_scales[:, None, None]).reshape(
        OUT, IN
    )
    wq8 = _quant_trn(wn)

    # w6[h, ot, p, k, ko, m] = wq8[o = OUT_C*h + 128*ot + m, i = 256k + 128ko + p]
    w6 = np.ascontiguousarray(
        wq8.T.reshape(KT, 2, 128, OUT_WAYS, OT, 128).transpose(3, 4, 2, 0, 1, 5)
    )
    alpha_full = (
        4.0 * in_scale.astype(np.float64) * w_scales.astype(np.float64)
    ).astype(np.float32)

    # x6[q][tt, p, k, ko, t] = xq8[token = T*q + NT*tt + t, i = 256k + 128ko + p]
    x6_by_q = [
        np.ascontiguousarray(
            xq8[T * q : T * (q + 1)]
            .reshape(TT, NT, KT, 2, 128)
            .transpose(0, 4, 2, 3, 1)
        )
        for q in range(TOKEN_WAYS)
    ]
    in_maps = []
    for cid in range(N_CORES):
        q, h = divmod(cid, OUT_WAYS)
        in_maps.append(
            {
                "x6": x6_by_q[q],
                "w5": w6[h],
                "biasv": bias[OUT_C * h : OUT_C * (h + 1)],
                "alpha": alpha_full[CHUNKS_C * h : CHUNKS_C * (h + 1)],
            }
        )
    return in_maps


def kernel(x, w, bias, in_scale, w_scales):
    nc = _build()
    in_maps = prepare_in_maps(x, w, bias, in_scale, w_scales)
    trace = bool(int(os.environ.get("TRN_KERNEL_TRACE", "0")))
    res = run_bass_kernel_spmd(nc, in_maps, list(range(N_CORES)), trace=trace)
    _CACHE["last_results"] = res

    out2d = np.empty((TOK, OUT), dtype=np.float32)
    for cid in range(N_CORES):
        q, h = divmod(cid, OUT_WAYS)
        out2d[T * q : T * (q + 1), OUT_C * h : OUT_C * (h + 1)] = res.results[cid][
            "outT"
        ].T
    return out2d.reshape(B, S, OUT)


# revision 5
# speedup vs baseline: 1.2006x; 1.2006x over previous
"""Bass/Trainium2 kernel for nn_DefaultSegmentLinear (fp8 segment linear).

Reference semantics (CHUNKS=4, seg_mode='weight'):
    xq = e4m3fn(clip(x / in_scale, +-448))          # OCP e4m3, max 448
    wq = e4m3fn(clip(w_c / w_scales[c], +-448))     # per out-chunk of 1024
    out = (xq @ wq_c^T) * in_scale * w_scales[c] + bias

Sharding: 4-way over the 16384 tokens x 2-way over the 4096 out
features (8 cores; core cid -> token quarter q=cid//2, out half
h=cid%2).

Quantization happens host-side, exactly on the reference's grid: round
to OCP e4m3fn (the reference's own quantize), then halve and re-round
to TRN e4m3 (max 240).  Every OCP-e4m3 point v <= 448 has v/2 <= 224
exactly representable in TRN e4m3 (up to deep subnormals, identical to
the on-device halved-quantize this replaces), and the 4x is folded into
the output scale alpha_c = 4*in_scale*w_scales[c].  The device then
runs pure double-pumped fp8 matmuls (perf_mode=DoubleRow, K=256 per
instruction) -- no on-device quantize pass, and input DMA drops from
96 MB f32 to 24 MB fp8 per core.

Per-core dataflow (contraction i on partitions for both operands):
    x6  [TT=8, 128, KT=16, 2, NT=512] fp8: token-block-major so the
        first matmul group only needs a 2 MB slice (~6 us) instead of
        the whole 16 MB of x -- kills the startup bubble.
    w5  [OT=16, 128, KT, 2, 128] fp8: all 16 o-tile weights resident in
        SBUF (64 KB/partition).
    outT [2048, 4096] f32 (o, t); host transposes back.

Loop: for each token block tt (512 tokens = 1 PSUM bank), for each
o-tile: 16 DoubleRow matmuls accumulate K=4096, then one DVE
tensor_scalar (psum*alpha + bias) and a DMA out.  PSUM banks rotate
across o-tiles so the PE never stalls.
"""

import os

import ml_dtypes
import numpy as np

import concourse.bacc as bacc
import concourse.mybir as mybir
from concourse import tile
from concourse.bass_utils import run_bass_kernel_spmd

N_CORES = 8
TOKEN_WAYS, OUT_WAYS = 4, 2
B, S, IN, OUT = 4, 4096, 4096, 4096
TOK = B * S
T = TOK // TOKEN_WAYS    # 4096 tokens per core
OUT_C = OUT // OUT_WAYS  # 2048 out features per core
KT = IN // 256           # 16 contraction super-tiles (256 = 128 x 2)
OT = OUT_C // 128        # 16 out-feature tiles per core
NT = 512                 # moving free dim per matmul (one PSUM bank of f32)
TT = T // NT             # 8 token blocks
CHUNKS = 4
CHUNKS_C = CHUNKS // OUT_WAYS  # 2 weight chunks per core
OT_PER_CHUNK = OT // CHUNKS_C  # 8

F32 = mybir.dt.float32
FP8 = mybir.dt.float8e4

E4M3FN = ml_dtypes.float8_e4m3fn  # OCP: max 448 (reference grid)
E4M3 = ml_dtypes.float8_e4m3      # IEEE/TRN: max 240

_CACHE = {}


def _build():
    key = "nc"
    if key in _CACHE:
        return _CACHE[key]
    nc = bacc.Bacc(None, target_bir_lowering=False)
    x6 = nc.dram_tensor("x6", [TT, 128, KT, 2, NT], FP8, kind="ExternalInput")
    w5 = nc.dram_tensor("w5", [OT, 128, KT, 2, 128], FP8, kind="ExternalInput")
    biasv = nc.dram_tensor("biasv", [OUT_C], F32, kind="ExternalInput")
    alpha = nc.dram_tensor("alpha", [CHUNKS_C], F32, kind="ExternalInput")
    outT = nc.dram_tensor("outT", [OUT_C, T], F32, kind="ExternalOutput")

    DR = mybir.MatmulPerfMode.DoubleRow

    with tile.TileContext(nc) as tc:
        with (
            tc.tile_pool(name="consts", bufs=1) as consts,
            tc.tile_pool(name="wq", bufs=1) as wqp,
            tc.tile_pool(name="xt", bufs=3) as xtp,
            tc.tile_pool(name="osb", bufs=4) as osbp,
            tc.tile_pool(name="psum", bufs=8, space="PSUM") as psp,
        ):
            # DMA order matters: the first matmuls gate on wq0 + the
            # leading k-chunk of x block 0, so those descriptors go
            # first.  x block 0 is split into 4 k-range chunks so real
            # matmuls can start after ~0.5 MB instead of 2 MB.
            wq = [
                wqp.tile([128, KT, 2, 128], FP8, tag=f"wq{ot}", name=f"wq{ot}")
                for ot in range(OT)
            ]
            nc.sync.dma_start(out=wq[0][:], in_=w5[0])
            xts = [None] * TT
            xts[0] = xtp.tile([128, KT, 2, NT], FP8, tag="xt", name="xt0")
            KC = KT // 4
            for kc in range(4):
                nc.sync.dma_start(
                    out=xts[0][:, KC * kc : KC * (kc + 1)],
                    in_=x6[0, :, KC * kc : KC * (kc + 1)],
                )

            al_b = []
            for c in range(CHUNKS_C):
                t = consts.tile([128, 1], F32, tag=f"al{c}")
                nc.sync.dma_start(
                    out=t[:], in_=alpha[c : c + 1].to_broadcast((128, 1))
                )
                al_b.append(t)
            bias_sb = consts.tile([128, OT], F32, tag="bias")
            nc.sync.dma_start(
                out=bias_sb[:], in_=biasv[:].rearrange("(j p) -> p j", p=128)
            )
            for ot in range(1, OT):
                nc.sync.dma_start(out=wq[ot][:], in_=w5[ot])

            # Warm-up: the PE's HAM clock gate needs ~3.4 us of
            # sustained busy to lift the 1.2 GHz cold throttle.  Run
            # garbage matmuls (wq0 against itself, scratch PSUM bank)
            # gated only on wq0's DMA so the array is at 2.4 GHz when
            # the first real group issues.
            warm_ps = psp.tile([128, NT], F32, tag="ps", name="warm")
            for i in range(48):
                nc.tensor.matmul(
                    warm_ps[:, :128],
                    lhsT=wq[0][:, 0, 0, :],
                    rhs=wq[0][:, 0, 0, :],
                    start=True,
                    stop=True,
                )

            for tt in range(TT):
                if xts[tt] is None:
                    xts[tt] = xtp.tile(
                        [128, KT, 2, NT], FP8, tag="xt", name=f"xt{tt}"
                    )
                    nc.sync.dma_start(out=xts[tt][:], in_=x6[tt])
                xt = xts[tt]
                # Prefetch next x block right after this one's DMA slot.
                nxt = tt + 1
                if nxt < TT and xts[nxt] is None:
                    xts[nxt] = xtp.tile(
                        [128, KT, 2, NT], FP8, tag="xt", name=f"xt{nxt}"
                    )
                    nc.sync.dma_start(out=xts[nxt][:], in_=x6[nxt])
                for ot in range(OT):
                    c = ot // OT_PER_CHUNK
                    ps = psp.tile([128, NT], F32, tag="ps", name=f"ps{tt}_{ot}")
                    for k in range(KT):
                        nc.tensor.matmul(
                            ps[:],
                            lhsT=wq[ot][:, k, :, :],
                            rhs=xt[:, k, :, :],
                            start=(k == 0),
                            stop=(k == KT - 1),
                            perf_mode=DR,
                        )
                    ob = osbp.tile([128, NT], F32, tag="osb", name=f"ob{tt}_{ot}")
                    nc.vector.tensor_scalar(
                        ob[:],
                        ps[:],
                        al_b[c][:],
                        bias_sb[:, ot : ot + 1],
                        op0=mybir.AluOpType.mult,
                        op1=mybir.AluOpType.add,
                    )
                    nc.sync.dma_start(
                        out=outT[
                            128 * ot : 128 * (ot + 1), NT * tt : NT * (tt + 1)
                        ],
                        in_=ob[:],
                    )
    nc.compile()
    _CACHE[key] = nc
    return nc


def _quant_trn(a):
    """Reference-grid quantize to TRN e4m3 at half scale.

    Round to OCP e4m3fn exactly as the reference does, then halve
    (exact in f32) and round to TRN/IEEE e4m3.  The second rounding is
    the identity except for deep subnormals (same as the on-device
    halved quantize this replaces)."""
    q = np.clip(a, -448.0, 448.0).astype(E4M3FN)
    return (q.astype(np.float32) * np.float32(0.5)).astype(E4M3)


def prepare_in_maps(x, w, bias, in_scale, w_scales):
    """Host-side prep: scale normalization, fp8 quantize, layout."""
    assert x.shape == (B, S, IN) and w.shape == (OUT, IN)
    x = np.ascontiguousarray(x, dtype=np.float32)
    w = np.ascontiguousarray(w, dtype=np.float32)
    bias = np.ascontiguousarray(bias, dtype=np.float32)
    in_scale = np.float32(np.asarray(in_scale).reshape(()))
    w_scales = np.asarray(w_scales, dtype=np.float32).reshape(CHUNKS)

    xq8 = _quant_trn(x.reshape(TOK, IN) / in_scale)
    wn = (w.reshape(CHUNKS, OUT // CHUNKS, IN) / w_scales[:, None, None]).reshape(
        OUT, IN
    )
    wq8 = _quant_trn(wn)

    # w6[h, ot, p, k, ko, m] = wq8[o = OUT_C*h + 128*ot + m, i = 256k + 128ko + p]
    w6 = np.ascontiguousarray(
        wq8.T.reshape(KT, 2, 128, OUT_WAYS, OT, 128).transpose(3, 4, 2, 0, 1, 5)
    )
    alpha_full = (
        4.0 * in_scale.astype(np.float64) * w_scales.astype(np.float64)
    ).astype(np.float32)

    # x6[q][tt, p, k, ko, t] = xq8[token = T*q + NT*tt + t, i = 256k + 128ko + p]
    x6_by_q = [
        np.ascontiguousarray(
            xq8[T * q : T * (q + 1)]
            .reshape(TT, NT, KT, 2, 128)
            .transpose(0, 4, 2, 3, 1)
        )
        for q in range(TOKEN_WAYS)
    ]
    in_maps = []
    for cid in range(N_CORES):
        q, h = divmod(cid, OUT_WAYS)
        in_maps.append(
            {
                "x6": x6_by_q[q],
                "w5": w6[h],
                "biasv": bias[OUT_C * h : OUT_C * (h + 1)],
                "alpha": alpha_full[CHUNKS_C * h : CHUNKS_C * (h + 1)],
            }
        )
    return in_maps


def kernel(x, w, bias, in_scale, w_scales):
    nc = _build()
    in_maps = prepare_in_maps(x, w, bias, in_scale, w_scales)
    trace = bool(int(os.environ.get("TRN_KERNEL_TRACE", "0")))
    res = run_bass_kernel_spmd(nc, in_maps, list(range(N_CORES)), trace=trace)
    _CACHE["last_results"] = res

    out2d = np.empty((TOK, OUT), dtype=np.float32)
    for cid in range(N_CORES):
        q, h = divmod(cid, OUT_WAYS)
        out2d[T * q : T * (q + 1), OUT_C * h : OUT_C * (h + 1)] = res.results[cid][
            "outT"
        ].T
    return out2d.reshape(B, S, OUT)
